# revision 2
# baseline (speedup 1.0000x reference)
"""Multi-head attention block kernel for Trainium2, sharded over 8 NeuronCores.

Sharding: batch (4) x head-group (2 groups of 8 heads) -> 8 cores.
Each core computes, for one batch b and one half of the heads:
  qh/kh/vh projections (columns of w_q/w_k/w_v for its heads),
  causal attention for its 8 heads, and a partial output projection
  (rows of w_o^T for its heads).  Host sums the two partial outputs per
  batch and transposes back.

On-chip layout is feature-major ("transposed"): activations live as
[feature, seq] so every matmul contraction dim is on partitions and no
on-chip transposes are needed.  Host pre-transposes q/k/v and the
weight slices, and post-transposes the output.

Matmuls run in bf16 (fp32 matmul is 4x slower on TRN2); accumulation is
fp32 in PSUM.  Softmax denominators come for free from an extra ones
column appended to each V tile (row 64 of the attn@V accumulator is the
sum of exp scores).

Phase 2 processes heads in even/odd pairs that live in PE-array row
halves 0:64 / 64:128: the two score matmuls carry tile_position (0,0)
and (64,0) and execute concurrently on HW (PE row tiling), the exp for
both heads is a single [128, 2*512] activation over two adjacent PSUM
banks, and diagonal (causally half-masked) tiles run exp/attn@V only on
the valid q range with a fixed 128x128 triangle mask.
"""

import sys

sys.path.insert(0, "/opt/trn_rl_repo")

import numpy as np
import ml_dtypes

import concourse.bacc as bacc
import concourse.mybir as mybir
import concourse.tile as tile
from concourse import bass_utils

B = 4
S = 2048
E = 1024
HEADS = 16
D = 64
H = 8            # heads per core
F = H * D        # 512 local head features
P = 128
ET = E // P      # 8 e-tiles
FT = F // P      # 4 f-tiles
ST = S // P      # 16 s-tiles
QC = 512         # q-chunk width
NQC = S // QC    # 4 q-chunks
KT_PER_QC = QC // P  # 4 k-tiles per q-chunk

BF16 = mybir.dt.bfloat16
F32 = mybir.dt.float32
NPBF16 = ml_dtypes.bfloat16
EXP = mybir.ActivationFunctionType.Exp


def build_nc(causal: bool, niter: int | None = None, phases=(1, 2, 3),
             no_norm=False, no_exp=False, xtlag=2, at_bufs=8,
             qcg_size=2, copy_split=True, diag_narrow=True):
    """Build the per-core Bass program.  If niter is given, wrap the body in a
    For_i timing loop (used by test.py to measure HW time)."""
    nc = bacc.Bacc("TRN2", target_bir_lowering=False, debug=False,
                   enable_asserts=True, num_devices=8)

    qT = nc.dram_tensor("qT", [E, S], BF16, kind="ExternalInput").ap()
    kT = nc.dram_tensor("kT", [E, S], BF16, kind="ExternalInput").ap()
    vT = nc.dram_tensor("vT", [E, S], BF16, kind="ExternalInput").ap()
    wqT = nc.dram_tensor("wqT", [E, F], BF16, kind="ExternalInput").ap()
    wkT = nc.dram_tensor("wkT", [E, F], BF16, kind="ExternalInput").ap()
    wvT = nc.dram_tensor("wvT", [E, F], BF16, kind="ExternalInput").ap()
    woT = nc.dram_tensor("woT", [F, E], BF16, kind="ExternalInput").ap()
    stair = nc.dram_tensor("stair", [P, 2 * QC], BF16, kind="ExternalInput").ap()
    if not causal:
        maskT = nc.dram_tensor("maskT", [S, S], BF16, kind="ExternalInput").ap()
    outT = nc.dram_tensor("outT", [E, S], F32, kind="ExternalOutput").ap()

    qT3 = qT.rearrange("(o p) s -> p o s", p=P)
    kT3 = kT.rearrange("(o p) s -> p o s", p=P)
    vT3 = vT.rearrange("(o p) s -> p o s", p=P)
    if not causal:
        maskT3 = maskT.rearrange("(o p) s -> p o s", p=P)

    with tile.TileContext(nc) as tc:
        import contextlib
        with contextlib.ExitStack() as ctx:
            persist = ctx.enter_context(tc.tile_pool(name="persist", bufs=1))
            streams = ctx.enter_context(tc.tile_pool(name="streams", bufs=5))
            attnp = ctx.enter_context(tc.tile_pool(name="attnp", bufs=at_bufs))
            smalls = ctx.enter_context(tc.tile_pool(name="smalls", bufs=3))
            # PSUM: scp (2 banks x 2 bufs) + 4 xt accumulators = 8 banks
            ps_sc = ctx.enter_context(tc.tile_pool(name="ps_sc", bufs=2, space="PSUM"))
            ps_xt = ctx.enter_context(tc.tile_pool(name="ps_xt", bufs=1, space="PSUM"))

            # Weights + constants: loaded once, outside the timing loop.
            wq_sb = persist.tile([P, ET, F], BF16, tag="wq")
            wk_sb = persist.tile([P, ET, F], BF16, tag="wk")
            wv_sb = persist.tile([P, ET, F], BF16, tag="wv")
            wo_sb = persist.tile([P, FT, E], BF16, tag="wo")
            stair_sb = persist.tile([P, 2 * QC], BF16, tag="stair")
            nc.sync.dma_start(wq_sb[:], wqT.rearrange("(o p) f -> p o f", p=P))
            nc.sync.dma_start(wk_sb[:], wkT.rearrange("(o p) f -> p o f", p=P))
            nc.sync.dma_start(wv_sb[:], wvT.rearrange("(o p) f -> p o f", p=P))
            nc.sync.dma_start(wo_sb[:], woT.rearrange("(o p) e -> p o e", p=P))
            nc.sync.dma_start(stair_sb[:], stair[:])
            # fixed 128x128 lower triangle: tri[kl, x] = (x >= kl)
            tri_sb = stair_sb[:, QC:QC + P]

            # Persistent activations (bf16): projections and attention outputs.
            qh_sb = persist.tile([P, FT, S], BF16, tag="qh")    # [f, ft, s]
            kh_sb = persist.tile([P, FT, S], BF16, tag="kh")
            vh_sb = persist.tile([P, ST, H, D + 1], BF16, tag="vh")  # ones col at d=64
            xts_sb = persist.tile([P, FT, S], BF16, tag="xts")

            copy_ctr = [0]

            def copy2(dst, src):
                # split PSUM->SBUF copies between DVE and ACT
                copy_ctr[0] += 1
                if copy_split and (copy_ctr[0] % 2 == 0):
                    nc.scalar.copy(dst, src)
                else:
                    nc.vector.tensor_copy(dst, src)

            def body():
                run1 = 1 in phases
                run2 = 2 in phases
                run3 = 3 in phases
                if not run1:
                    nc.vector.memset(qh_sb[:, :, 0:1], 0.5)
                    nc.vector.memset(kh_sb[:, :, 0:1], 0.5)
                    nc.vector.memset(vh_sb[:, :, :, 0:1], 0.5)
                if not run2 and run3:
                    nc.vector.memset(xts_sb[:, :, 0:1], 0.5)

                # ---- Phase 1a: q/k projections -> qh/kh (feature-major) ----
                # Weight-stationary: for each (ft, et) weight tile, stream all
                # 4 s-chunks into 4 accumulating PSUM banks (2 scp pair-tiles)
                # so LDWEIGHTS happens once per 4 matmuls.
                for src3, w_sb, dst in ((qT3, wq_sb, qh_sb), (kT3, wk_sb, kh_sb)) if run1 else ():
                    xcs = []
                    for sc in range(NQC):
                        xc = streams.tile([P, ET, QC], BF16, tag="xc")
                        nc.sync.dma_start(xc[:], src3[:, :, sc * QC:(sc + 1) * QC])
                        xcs.append(xc)
                    for ft in range(FT):
                        scps = [ps_sc.tile([P, 2, QC], F32, tag="scp", name=f"p1{g}")
                                for g in range(2)]
                        for et in range(ET):
                            for sc in range(NQC):
                                nc.tensor.matmul(
                                    scps[sc // 2][:, sc % 2, :],
                                    w_sb[:, et, ft * P:(ft + 1) * P],
                                    xcs[sc][:, et, :],
                                    start=(et == 0), stop=(et == ET - 1))
                        for g in range(2):
                            copy2(
                                dst[:, ft, g * 2 * QC:(g + 1) * 2 * QC]
                                .rearrange("p (a b) -> p a b", a=2),
                                scps[g][:])

                # ---- Phase 1b: v projection -> vh (seq-major) + ones column ----
                nc.vector.memset(vh_sb[:, :, :, D:D + 1], 1.0)
                for sc in range(NQC) if run1 else ():
                    xc = streams.tile([P, ET, QC], BF16, tag="xc")
                    nc.sync.dma_start(xc[:], vT3[:, :, sc * QC:(sc + 1) * QC])
                    for g in range(2):
                        scp = ps_sc.tile([P, 2, QC], F32, tag="scp", name="pv")
                        for i in range(2):
                            si = g * 2 + i
                            for et in range(ET):
                                nc.tensor.matmul(
                                    scp[:, i, :],
                                    xc[:, et, si * P:(si + 1) * P],
                                    wv_sb[:, et, :],
                                    start=(et == 0), stop=(et == ET - 1))
                        st0 = sc * KT_PER_QC + g * 2
                        copy2(vh_sb[:, st0:st0 + 2, :, 0:D],
                              scp[:].rearrange("p a (h d) -> p a h d", h=H))

                # ---- Phase 2: attention ----
                def normalize(xt_psum, h, qc):
                    ft, fo = h // 2, (h % 2) * D
                    if no_norm:
                        nc.vector.tensor_copy(
                            xts_sb[fo:fo + D, ft, qc * QC:(qc + 1) * QC],
                            xt_psum[0:D, :])
                    else:
                        recip = smalls.tile([1, QC], F32, tag="recip")
                        nc.vector.reciprocal(recip[:], xt_psum[D:D + 1, :])
                        rb = smalls.tile([D, QC], F32, tag="rb")
                        nc.gpsimd.partition_broadcast(rb[:], recip[0:1, :])
                        nc.vector.tensor_mul(
                            xts_sb[fo:fo + D, ft, qc * QC:(qc + 1) * QC],
                            xt_psum[0:D, :], rb[:])

                if run2 and causal:
                    # Head-pair processing: heads (2*hp, 2*hp+1) live in
                    # partition halves 0:64 / 64:128 of f-tile hp.  Score
                    # matmuls for the pair are row-tiled (concurrent on PE);
                    # exp covers both heads' PSUM banks in one activation.
                    # kt-outer within a qc-group so stationary K/V tiles are
                    # loaded once per (pair, kt) and streamed over the group.
                    NG = NQC // qcg_size
                    for hp in range(4):
                        for qcg in range(NG):
                            qcs_g = list(range(qcg * qcg_size, (qcg + 1) * qcg_size))
                            ktm = (qcs_g[-1] + 1) * KT_PER_QC
                            xt_ps = {}
                            for qc in qcs_g:
                                for par in (0, 1):
                                    t = ps_xt.tile(
                                        [D + 1, QC], F32,
                                        tag=f"xt{qc % qcg_size}{par}",
                                        name=f"xt{qc % qcg_size}{par}")
                                    xt_ps[(qc, par)] = t
                            pend = []  # [(kt, [(qc, at, off)])]

                            def flush(gen):
                                kt2, items = gen
                                for qc, at, off in items:
                                    for par in (0, 1):
                                        nc.tensor.matmul(
                                            xt_ps[(qc, par)][:, off:],
                                            vh_sb[:, kt2, 2 * hp + par, :],
                                            at[:, par, off:],
                                            start=(kt2 == 0),
                                            stop=(kt2 == (qc + 1) * KT_PER_QC - 1))

                            for kt in range(ktm):
                                items = []
                                for qc in qcs_g:
                                    if kt >= (qc + 1) * KT_PER_QC:
                                        continue
                                    scp = ps_sc.tile([P, 2, QC], F32, tag="scp",
                                                     name="scp")
                                    for par in (0, 1):
                                        nc.tensor.matmul(
                                            scp[:, par, :],
                                            kh_sb[par * D:(par + 1) * D, hp,
                                                  kt * P:(kt + 1) * P],
                                            qh_sb[par * D:(par + 1) * D, hp,
                                                  qc * QC:(qc + 1) * QC],
                                            start=True, stop=True)
                                    at = attnp.tile([P, 2, QC], BF16, tag="at",
                                                    name="at")
                                    diag = (kt // KT_PER_QC == qc)
                                    off = (kt % KT_PER_QC) * P if (diag and diag_narrow) else 0
                                    if no_exp:
                                        nc.vector.tensor_copy(
                                            at[:, :, off:], scp[:, :, off:])
                                    else:
                                        nc.scalar.activation(
                                            at[:, :, off:], scp[:, :, off:],
                                            EXP, scale=0.125)
                                    if diag:
                                        o2 = (kt % KT_PER_QC) * P
                                        for par in (0, 1):
                                            nc.vector.tensor_mul(
                                                at[:, par, o2:o2 + P],
                                                at[:, par, o2:o2 + P],
                                                tri_sb)
                                        if not diag_narrow and o2 > 0:
                                            # zero the fully-masked region
                                            nc.vector.memset(at[:, :, 0:o2], 0.0)
                                    items.append((qc, at, off))
                                pend.append((kt, items))
                                if len(pend) > xtlag:
                                    flush(pend.pop(0))
                            for gen in pend:
                                flush(gen)
                            for qc in qcs_g:
                                for par in (0, 1):
                                    normalize(xt_ps[(qc, par)], 2 * hp + par, qc)

                elif run2:
                    # general-mask path: qc-outer, mask tiles streamed per qc.
                    for qc in range(NQC):
                        mc = streams.tile([P, ST, QC], BF16, tag="mc")
                        nc.sync.dma_start(mc[:], maskT3[:, :, qc * QC:(qc + 1) * QC])
                        ktm = ST
                        for h in range(H):
                            ft, fo = h // 2, (h % 2) * D
                            xt_psum = ps_xt.tile([D + 1, QC], F32, tag="xt00")
                            at_tiles = [None] * ktm

                            def emit_sc(kt):
                                scp = ps_sc.tile([P, 2, QC], F32, tag="scp")
                                nc.tensor.matmul(
                                    scp[:, 0, :],
                                    kh_sb[fo:fo + D, ft, kt * P:(kt + 1) * P],
                                    qh_sb[fo:fo + D, ft, qc * QC:(qc + 1) * QC],
                                    start=True, stop=True)
                                at = attnp.tile([P, 2, QC], BF16, tag="at")
                                if no_exp:
                                    nc.vector.tensor_copy(at[:, 0, :], scp[:, 0, :])
                                else:
                                    nc.scalar.activation(at[:, 0, :], scp[:, 0, :],
                                                         EXP, scale=0.125)
                                nc.vector.tensor_mul(at[:, 0, :], at[:, 0, :],
                                                     mc[:, kt, :])
                                at_tiles[kt] = at

                            def emit_xt(kt):
                                nc.tensor.matmul(
                                    xt_psum[:],
                                    vh_sb[:, kt, h, :],
                                    at_tiles[kt][:, 0, :],
                                    start=(kt == 0), stop=(kt == ktm - 1))

                            PIPE = 2
                            for kt in range(ktm):
                                emit_sc(kt)
                                if kt >= PIPE:
                                    emit_xt(kt - PIPE)
                            for kt in range(max(0, ktm - PIPE), ktm):
                                emit_xt(kt)
                            normalize(xt_psum, h, qc)

                # ---- Phase 3: output projection (partial over local heads) ----
                for jt in range(ET) if run3 else ():
                    scps = [ps_sc.tile([P, 2, QC], F32, tag="scp", name=f"p3{g}")
                            for g in range(2)]
                    for ft in range(FT):
                        for qc in range(NQC):
                            nc.tensor.matmul(
                                scps[qc // 2][:, qc % 2, :],
                                wo_sb[:, ft, jt * P:(jt + 1) * P],
                                xts_sb[:, ft, qc * QC:(qc + 1) * QC],
                                start=(ft == 0), stop=(ft == FT - 1))
                    for g in range(2):
                        ot = streams.tile([P, 2 * QC], F32, tag="ot")
                        nc.vector.tensor_copy(
                            ot[:].rearrange("p (a b) -> p a b", a=2), scps[g][:])
                        nc.sync.dma_start(
                            outT[jt * P:(jt + 1) * P, g * 2 * QC:(g + 1) * 2 * QC],
                            ot[:])

            if niter is None:
                body()
            else:
                with tc.For_i(0, niter, 1):
                    body()

    nc.compile()
    return nc


def _host_prep(q, k, v, mask, w_q, w_k, w_v, w_o):
    """Shard + transpose inputs on the host.  Returns (in_maps, causal)."""
    tril = np.tril(np.ones((S, S), dtype=mask.dtype))
    causal = all(np.array_equal(np.asarray(mask[b, 0]), tril) for b in range(B))

    stair = (np.arange(2 * QC)[None, :] >= (np.arange(P)[:, None] + QC))
    stair = stair.astype(NPBF16)

    w_q = np.asarray(w_q, dtype=np.float32)
    w_k = np.asarray(w_k, dtype=np.float32)
    w_v = np.asarray(w_v, dtype=np.float32)
    w_o = np.asarray(w_o, dtype=np.float32)

    in_maps = []
    for core in range(8):
        b, g = divmod(core, 2)
        rows = slice(g * F, (g + 1) * F)
        m = {
            "qT": np.ascontiguousarray(np.asarray(q[b], np.float32).T).astype(NPBF16),
            "kT": np.ascontiguousarray(np.asarray(k[b], np.float32).T).astype(NPBF16),
            "vT": np.ascontiguousarray(np.asarray(v[b], np.float32).T).astype(NPBF16),
            "wqT": np.ascontiguousarray(w_q[rows, :].T).astype(NPBF16),
            "wkT": np.ascontiguousarray(w_k[rows, :].T).astype(NPBF16),
            "wvT": np.ascontiguousarray(w_v[rows, :].T).astype(NPBF16),
            "woT": np.ascontiguousarray(w_o[:, rows].T).astype(NPBF16),
            "stair": stair,
        }
        if not causal:
            m["maskT"] = np.ascontiguousarray(
                np.asarray(mask[b, 0], np.float32).T).astype(NPBF16)
        in_maps.append(m)
    return in_maps, causal


_NC_CACHE: dict = {}


def kernel(q, k, v, mask, w_q, w_k, w_v, w_o):
    in_maps, causal = _host_prep(q, k, v, mask, w_q, w_k, w_v, w_o)
    nc = _NC_CACHE.get(causal)
    if nc is None:
        nc = build_nc(causal)
        _NC_CACHE[causal] = nc
    res = bass_utils.run_bass_kernel_spmd(nc, in_maps, core_ids=list(range(8)))
    out = np.empty((B, S, E), dtype=np.float32)
    for b in range(B):
        out[b] = (res.results[2 * b]["outT"] + res.results[2 * b + 1]["outT"]).T
    return out
